# revision 4
# baseline (speedup 1.0000x reference)
"""Trainium2 Bass kernel for nn_CatAttention — v2.

Changes vs v1:
  - Causal trim: score matmuls + Ln/Exp only cover p <= q at 128-block
    granularity (75% -> 56% of full score volume), with score strips
    packed tightly into PSUM groups so Ln instruction count stays low.
  - Optional bf16 operands for all matmuls (FWL weight loads, half DMA).
  - x is staged per 512-qi chunk contiguously (1 DMA per half).

Math identical to v1 (see v1 docstring): attn = softmax over valid p of
(log(S)+bias)/8 = S^{1/8} * exp(bias/8) normalized; bias factors into a
per-p exp(p/8184) (Exp bias + per-tile constant on v) and a diagonal
correction RHO; relu-correction to pre[...,0] is below f32 resolution.

Sharding: core c = batch c//2, heads 4*(c%2)..4*(c%2)+3 (2 workgroups
of 2 heads stacked on the partition axis).
"""
import math
import numpy as np

BATCH, N_CTX, D_IN, N_HEADS, D_HEAD, N_VARS = 4, 1024, 512, 8, 64, 8
P = 128
NKT = D_IN // P           # 4 d_in tiles
NPT = N_CTX // P          # 8 p-tiles
QCW = 512                 # qi chunk width
NQC = N_CTX // QCW        # 2 qi chunks
INV8184 = 1.0 / (1023.0 * 8.0)
RHO = float(np.exp(np.float64(-2.0 / 8.0) - np.float64(1.0 / 1023.0 / 8.0)))
VA = D_HEAD + 1           # v columns + ones column

USE_BF16 = True

# Packed score groups per qi-chunk: list of (psum_cols, [(pt_local,
# qi_off, width, col), ...]).  pt_local is the p-tile index relative to
# the chunk's p range (pt_global = pt_local for qc0/qc1 since both
# start at p-tile 0).  qi_off/width give the causal trim; col is the
# packed column inside the PSUM group.  Every matmul [col, col+width)
# stays inside one 512-col PSUM bank.
GROUPS = {
    0: [
        (1536, [(0, 0, 512, 0), (1, 128, 384, 512), (3, 384, 128, 896),
                (2, 256, 256, 1024)]),
    ],
    1: [
        (1536, [(0, 0, 512, 0), (1, 0, 512, 512), (2, 0, 512, 1024)]),
        (1024, [(3, 0, 512, 0), (4, 0, 512, 512)]),
        (768, [(5, 128, 384, 0), (7, 384, 128, 384), (6, 256, 256, 512)]),
    ],
}
# packed e length per chunk and (pt -> (qi_off, packed col in e)) maps
E_LEN = {0: 1280, 1: 3328}
E_POS = {}
for _qc, _gs in GROUPS.items():
    _off = 0
    _pos = {}
    _plen = 0
    for _cols, _items in _gs:
        for _pt, _qoff, _w, _c in _items:
            _pos[_pt] = (_qoff, _plen + _c if False else None)
        _plen += 0
    # packed e concatenates the USED part of each group back-to-back
    _base = 0
    for _cols, _items in _gs:
        _used = max(_c + _w for _pt, _qoff, _w, _c in _items)
        for _pt, _qoff, _w, _c in _items:
            _pos[_pt] = (_qoff, _base + _c)
        _base += _used
    assert _base == E_LEN[_qc], (_qc, _base)
    E_POS[_qc] = _pos

_COMPILED = {}


def _softmax_f32(w):
    w = np.asarray(w, dtype=np.float32)
    m = w.max(axis=-1, keepdims=True)
    e = np.exp(w - m, dtype=np.float32)
    return e / e.sum(axis=-1, keepdims=True, dtype=np.float32)


def _host_weights(W_K_W, W_Q_W, W_V_W, W_pred_W):
    """Fold ConstrainedRead + WPred into dense (d_in, 64) mats per head."""
    probsK = _softmax_f32(W_K_W)
    probsQ = _softmax_f32(W_Q_W)
    probsV = _softmax_f32(W_V_W)
    Wp = _softmax_f32(W_pred_W)
    eye = np.eye(D_HEAD, dtype=np.float32)
    WK = np.stack([np.kron(probsK[h][:, None], eye) for h in range(N_HEADS)])
    WQm = np.stack([np.kron(probsQ[h][:, None], eye) for h in range(N_HEADS)])
    WQ = np.einsum('hde,hef->hdf', WQm, Wp).astype(np.float32)
    WV = np.stack([np.kron(probsV[h][:, None], eye) for h in range(N_HEADS)])
    return WK, WQ, WV   # each (8, 512, 64)


def _stack_wg(W, h0, nh=2):
    """nh heads of (512,64) -> SBUF layout (128, 4, nh*64): [i, kt, m]."""
    s = np.concatenate([W[h0 + j] for j in range(nh)], axis=1)
    return np.ascontiguousarray(s.reshape(NKT, P, nh * D_HEAD).transpose(1, 0, 2))


def _gdiag():
    """GD[i,u] = h(u-127-i); h(d<0)=0, h(0)=RHO, h(d>0)=1."""
    i = np.arange(P)[:, None]
    u = np.arange(2 * P)[None, :]
    d = u - (P - 1) - i
    g = np.where(d < 0, 0.0, np.where(d == 0, RHO, 1.0))
    return np.ascontiguousarray(g.astype(np.float32))


def _pinned_bacc_cls():
    """Bacc pinning the ACT table set containing both Ln and Exp."""
    import concourse.bacc as bacc
    import concourse.mybir as mybir
    import bass_rust as _bass_rust
    from concourse.hw_specs import get_activation_tables

    class _PinnedActBacc(bacc.Bacc):
        def insert_act_table_loads(self):
            has_activation = any(
                isinstance(i, mybir.InstActivation)
                for b in self.main_func.blocks for i in b.instructions)
            if not has_activation:
                return
            tables = [
                (name, fns if name == "natural_log_exp_and_others" else set())
                for name, fns in get_activation_tables(self.m.arch).items()
            ]
            _bass_rust.insert_act_table_loads(self, tables)

    return _PinnedActBacc


def _build_nc(reps=1, barrier=True):
    import concourse.mybir as mybir
    import concourse.tile as tile
    from contextlib import ExitStack

    F32 = mybir.dt.float32
    F32R = mybir.dt.float32r
    BF16 = mybir.dt.bfloat16
    MMT = BF16 if USE_BF16 else F32R
    IN_T = BF16 if USE_BF16 else F32

    def rr(ap):
        return ap if USE_BF16 else ap.bitcast(F32R)

    nc = _pinned_bacc_cls()("TRN2")
    xT_d = nc.dram_tensor("xT", (2, P, NKT, QCW), IN_T, kind="ExternalInput")
    WKQ_d = nc.dram_tensor("WKQ", (2, P, NKT, 2 * P), IN_T, kind="ExternalInput")
    WV_d = nc.dram_tensor("WV", (P, NKT, 4 * D_HEAD), IN_T, kind="ExternalInput")
    GDRB_d = nc.dram_tensor("GDRB", (P, 2 * P + 1), F32, kind="ExternalInput")
    out_d = nc.dram_tensor("out", (N_CTX, 4 * D_HEAD), IN_T, kind="ExternalOutput")

    LN = mybir.ActivationFunctionType.Ln
    EXP = mybir.ActivationFunctionType.Exp

    with tile.TileContext(nc) as tc, ExitStack() as ctx:
        const_p = ctx.enter_context(tc.tile_pool(name="const", bufs=1))
        w_p = ctx.enter_context(tc.tile_pool(name="w", bufs=2))
        kq_p = ctx.enter_context(tc.tile_pool(name="kq", bufs=2))
        va_p = ctx.enter_context(tc.tile_pool(name="va", bufs=2))
        e_p = ctx.enter_context(tc.tile_pool(name="e", bufs=3))
        z_p = ctx.enter_context(tc.tile_pool(name="z", bufs=4))
        # scores PSUM: groups are <=3 banks; bufs=2 -> <=6 banks live
        ps_s = ctx.enter_context(tc.tile_pool(name="ps_s", bufs=2, space="PSUM"))
        # small PSUM for projections / v / mm2 / warmup: 1 bank x 2
        ps_1 = ctx.enter_context(tc.tile_pool(name="ps_1", bufs=2, space="PSUM"))

        gdrb = const_p.tile([P, 2 * P + 1], F32, tag="gdrb")
        gd32 = gdrb[:, 0:2 * P]
        rb = gdrb[:, 2 * P:2 * P + 1]
        gd = const_p.tile([P, P], MMT, tag="gd16")
        eps = const_p.tile([P, 1], F32, tag="eps")
        nc.vector.memset(eps[:], 1e-20)
        warm = const_p.tile([1, QCW], MMT, tag="warm")
        nc.vector.memset(warm[:], 0.0)

        mm2q = []
        zstages = {}

        def emit_mm2_jt(item):
            e, wg, hh, qc, vaug, jl = item
            key = (wg, qc)
            if key not in zstages:
                zstages[key] = z_p.tile([P, 4, 2 * D_HEAD], IN_T, tag="zst",
                                        name=f"zst_{wg}_{qc}")
            zst = zstages[key]
            jt = qc * 4 + jl
            zps = ps_1.tile([P, 512], F32, tag="ps")
            pos = E_POS[qc]
            for pt in range(jt + 1):
                qoff, base = pos[pt]
                c0 = base + (jl * P - qoff)
                nc.tensor.matmul(
                    zps[:, 0:VA],
                    e[:, c0:c0 + P],
                    vaug[:, pt * 4 + wg * 2 + hh, :],
                    start=(pt == 0), stop=(pt == jt))
            rcp = z_p.tile([P, 1], F32, tag="rcp")
            nc.vector.reciprocal(rcp[:], zps[:, D_HEAD:VA])
            nc.vector.tensor_scalar_mul(
                zst[:, jl, hh * D_HEAD:(hh + 1) * D_HEAD],
                zps[:, 0:D_HEAD], rcp[:])
            if wg == 1 and qc == 0 and hh == 1 and jl in (1, 3):
                # tail unit: ship each half as soon as its rows are done,
                # second half on the (now idle) ACT ring — hides the HBM
                # write-receipt latency of the final transfer
                j0, j1 = (0, 2) if jl == 1 else (2, 4)
                dst = out_d[j0 * P:j1 * P,
                            wg * 2 * D_HEAD:(wg + 1) * 2 * D_HEAD]
                eng = nc.sync if jl == 1 else nc.scalar
                eng.dma_start(dst.rearrange("(j p) c -> p j c", p=P),
                              zst[:, j0:j1, :])
                if jl == 3:
                    del zstages[key]
            elif hh == 1 and jl == 3:
                dst = out_d[qc * QCW:(qc + 1) * QCW,
                            wg * 2 * D_HEAD:(wg + 1) * 2 * D_HEAD]
                nc.sync.dma_start(dst.rearrange("(j p) c -> p j c", p=P), zst[:])
                del zstages[key]

        def drain_mm2(keep):
            while len(mm2q) > keep:
                emit_mm2_jt(mm2q.pop(0))

        for rep in range(reps):
          if rep and barrier:
              tc.strict_bb_all_engine_barrier()
          # PE warmup during the input-DMA wait (HAM clock ramp)
          wps = ps_1.tile([P, 512], F32, tag="ps")
          for _ in range(3):
              nc.tensor.matmul(wps[0:1, 0:QCW], warm[0:1, 0:1], warm[0:1, :],
                               start=True, stop=True)
          nc.vector.tensor_add(eps[0:1, 0:1], eps[0:1, 0:1], wps[0:1, 0:1])
          wkq0 = w_p.tile([P, NKT, 2 * P], MMT, tag="wkq")
          nc.sync.dma_start(wkq0[:], rr(WKQ_d[0]))
          xA = const_p.tile([P, NKT, QCW], MMT, tag="xA")
          xB = const_p.tile([P, NKT, QCW], MMT, tag="xB")
          # split the ch0 x load across both HWDGE rings (ACT ring is idle
          # this early) so the first projection's input lands sooner
          nc.scalar.dma_start(xA[:, 0:2, :], rr(xT_d[0][:, 0:2, :]))
          nc.sync.dma_start(xA[:, 2:4, :], rr(xT_d[0][:, 2:4, :]))
          if rep == 0:
              nc.sync.dma_start(gdrb[:], GDRB_d[:])
              nc.vector.tensor_copy(gd[:], gd32[:, P - 1:2 * P - 1])
          xhalf = [xA, xB]

          def xs(kt, col, width):
              t = xhalf[col // QCW]
              c = col % QCW
              return t[:, kt, c:c + width]

          wv = w_p.tile([P, NKT, 4 * D_HEAD], MMT, tag="wv")
          vaug = va_p.tile([P, NPT * 4, VA], MMT, tag="vaug")
          vaug_dma_done = [False]

          def emit_vaug(pts):
              if not vaug_dma_done[0]:
                  nc.sync.dma_start(wv[:], rr(WV_d[:]))
                  for pt in range(NPT):
                      c_pt = float(math.exp(P * pt * INV8184))
                      nc.vector.memset(
                          vaug[:, pt * 4:(pt + 1) * 4, D_HEAD:VA], c_pt)
                  vaug_dma_done[0] = True
              for pt in pts:
                  vps = ps_1.tile([P, 512], F32, tag="ps")
                  for kt in range(NKT):
                      nc.tensor.matmul(vps[:, 0:4 * D_HEAD],
                                       xs(kt, pt * P, P),
                                       wv[:, kt, :],
                                       start=(kt == 0), stop=(kt == NKT - 1))
                  c_pt = float(math.exp(P * pt * INV8184))
                  nc.vector.tensor_scalar_mul(
                      vaug[:, pt * 4:(pt + 1) * 4, 0:D_HEAD],
                      vps[:, 0:4 * D_HEAD].rearrange("p (a b) -> p a b", a=4),
                      c_pt)

          for wg in range(2):
            if wg == 0:
                wkq = wkq0
                nc.scalar.dma_start(xB[:], rr(xT_d[1]))
            else:
                wkq = w_p.tile([P, NKT, 2 * P], MMT, tag="wkq")
                nc.sync.dma_start(wkq[:], rr(WKQ_d[wg]))
            wk = wkq[:, :, 0:P]
            wq = wkq[:, :, P:2 * P]

            # kT2/qT2: [128 = 2 heads x 64 dh, 1024 p/qi]
            kt2 = kq_p.tile([P, N_CTX], MMT, tag="kt2")
            qt2 = kq_p.tile([P, N_CTX], MMT, tag="qt2")

            def emit_proj(ch):
                cs = slice(ch * QCW, (ch + 1) * QCW)
                kps = ps_1.tile([P, 512], F32, tag="ps")
                for kt in range(NKT):
                    nc.tensor.matmul(kps[:, 0:QCW], wk[:, kt, :],
                                     xs(kt, ch * QCW, QCW),
                                     start=(kt == 0), stop=(kt == NKT - 1))
                nc.vector.tensor_copy(kt2[:, cs], kps[:, 0:QCW])
                qps = ps_1.tile([P, 512], F32, tag="ps")
                for kt in range(NKT):
                    nc.tensor.matmul(qps[:, 0:QCW], wq[:, kt, :],
                                     xs(kt, ch * QCW, QCW),
                                     start=(kt == 0), stop=(kt == NKT - 1))
                nc.vector.tensor_copy(qt2[:, cs], qps[:, 0:QCW])

            emit_proj(0)
            if wg == 1:
                emit_proj(1)

            # wg0 runs both qc0 units first: they need only the ch0
            # projections, so ACT chews on them while PE does the ch1
            # projections + v, hiding the xB/proj latency.
            units = ([(0, 0), (1, 0), (0, 1), (1, 1)] if wg == 0
                     else [(0, 0), (0, 1), (1, 1), (1, 0)])
            for hh, qc in units:
                    hb = hh * D_HEAD
                    qbase = qc * QCW
                    e = e_p.tile([P, E_LEN[qc]], MMT, tag="e",
                                 name=f"e_{wg}_{hh}_{qc}")
                    ebase = 0
                    for cols, items in GROUPS[qc]:
                        sps = ps_s.tile([P, cols], F32, tag="ps")
                        used = max(c + w for _, _, w, c in items)
                        for pt, qoff, w, c in items:
                            nc.tensor.matmul(
                                sps[:, c:c + w],
                                kt2[hb:hb + D_HEAD, pt * P:(pt + 1) * P],
                                qt2[hb:hb + D_HEAD,
                                    qbase + qoff:qbase + qoff + w],
                                start=True, stop=True)
                        nc.scalar.activation(e[:, ebase:ebase + used],
                                             sps[:, 0:used], LN, bias=eps[:])
                        ebase += used
                    last_unit = wg == 1 and hh == 1 and qc == 0
                    pos = E_POS[qc]
                    if last_unit:
                        # tail overlap: flush older attn@v, then Exp per
                        # p-strip and interleave this unit's attn@v
                        drain_mm2(0)
                        for jl in range(4):
                            qoff, base = pos[jl]
                            w = QCW - qoff
                            nc.scalar.activation(e[:, base:base + w],
                                                 e[:, base:base + w],
                                                 EXP, bias=rb, scale=0.125)
                            dpos = base + (jl * P - qoff)
                            nc.vector.tensor_mul(e[:, dpos:dpos + P],
                                                 e[:, dpos:dpos + P], gd[:])
                            emit_mm2_jt((e, wg, hh, qc, vaug, jl))
                        continue
                    nc.scalar.activation(e[:], e[:], EXP, bias=rb, scale=0.125)
                    for jl in range(4):
                        pt = qc * 4 + jl
                        qoff, base = pos[pt]
                        dpos = base + (jl * P - qoff)
                        nc.vector.tensor_mul(e[:, dpos:dpos + P],
                                             e[:, dpos:dpos + P], gd[:])
                    if wg == 0 and hh == 1 and qc == 0:
                        # both qc0 units done: ch1 projections + v for the
                        # qc0 attn@v now, while ACT runs the queued Ln/Exp
                        emit_proj(1)
                        emit_vaug(range(0, 4))
                    if wg == 0 and hh == 0 and qc == 1:
                        emit_vaug(range(4, NPT))
                    if not (wg == 0 and hh == 0 and qc == 0):
                        drain_mm2(2)
                    for jl in range(4):
                        mm2q.append((e, wg, hh, qc, vaug, jl))
          drain_mm2(0)
    nc.finalize()
    return nc


def _get_nc(reps=1, barrier=True):
    key = (reps, barrier)
    if key not in _COMPILED:
        _COMPILED[key] = _build_nc(reps, barrier)
    return _COMPILED[key]


def _np_cast(a):
    if not USE_BF16:
        return np.ascontiguousarray(a, dtype=np.float32)
    import ml_dtypes
    return np.ascontiguousarray(a.astype(ml_dtypes.bfloat16))


def _make_in_maps(x, WK, WQ, WV):
    gdrb = np.concatenate([
        _gdiag(),
        np.arange(P, dtype=np.float32)[:, None] * np.float32(INV8184),
    ], axis=1).astype(np.float32)
    in_maps = []
    for c in range(8):
        b, hg = c // 2, c % 2
        h0 = hg * 4
        xTh = x[b].T.reshape(NKT, P, N_CTX).transpose(1, 0, 2)  # [128,4,1024]
        xAB = np.stack([xTh[:, :, 0:QCW], xTh[:, :, QCW:N_CTX]])
        wkq = [np.concatenate([_stack_wg(WK, h), _stack_wg(WQ, h)], axis=2)
               for h in (h0, h0 + 2)]
        in_maps.append({
            "xT": _np_cast(xAB),
            "WKQ": _np_cast(np.stack(wkq)),
            "WV": _np_cast(_stack_wg(WV, h0, nh=4)),
            "GDRB": np.ascontiguousarray(gdrb),
        })
    return in_maps


def _make_runner(nc, in_maps):
    """Reusable jitted 8-core runner (no donation, device-resident inputs)."""
    import jax
    from jax.sharding import Mesh, NamedSharding, PartitionSpec
    from jax.experimental.shard_map import shard_map
    import concourse.bass2jax as b2j
    import concourse.mybir as mybir

    b2j.install_neuronx_cc_hook()
    partition_name = nc.partition_id_tensor.name if nc.partition_id_tensor else None
    in_names, out_names, out_avals, zero_outs = [], [], [], []
    for alloc in nc.m.functions[0].allocations:
        if not isinstance(alloc, mybir.MemoryLocationSet):
            continue
        name = alloc.memorylocations[0].name
        if alloc.kind == "ExternalInput":
            if name != partition_name:
                in_names.append(name)
        elif alloc.kind == "ExternalOutput":
            out_names.append(name)
            shape = tuple(alloc.tensor_shape)
            dtype = mybir.dt.np(alloc.dtype)
            out_avals.append(jax.core.ShapedArray(shape, dtype))
            zero_outs.append(np.zeros(shape, dtype))
    n_params = len(in_names)
    all_in = in_names + out_names + ([partition_name] if partition_name else [])

    def _body(*args):
        operands = list(args)
        if partition_name:
            operands.append(b2j.partition_id_tensor())
        outs = b2j._bass_exec_p.bind(
            *operands, out_avals=tuple(out_avals), in_names=tuple(all_in),
            out_names=tuple(out_names), lowering_input_output_aliases=(),
            sim_require_finite=True, sim_require_nnan=True, nc=nc)
        return tuple(outs)

    n_cores = 8
    devices = jax.devices()[:n_cores]
    mesh = Mesh(np.asarray(devices), ("core",))
    nspec = n_params + len(out_names)
    fn = jax.jit(
        shard_map(_body, mesh=mesh, in_specs=(PartitionSpec("core"),) * nspec,
                  out_specs=(PartitionSpec("core"),) * len(out_names),
                  check_rep=False),
        keep_unused=True)
    concat_in = [np.concatenate([np.asarray(in_maps[c][nm]) for c in range(n_cores)],
                                axis=0) for nm in in_names]
    concat_zeros = [np.zeros((n_cores * z.shape[0], *z.shape[1:]), z.dtype)
                    for z in zero_outs]
    sh = NamedSharding(mesh, PartitionSpec("core"))
    args = [jax.device_put(a, sh) for a in concat_in + concat_zeros]

    def run():
        outs = fn(*args)
        jax.block_until_ready(outs)
        return outs
    return run, out_names, out_avals


def _mask_is_tril(mask):
    mask = np.asarray(mask)
    tril = np.tril(np.ones((N_CTX, N_CTX), dtype=bool))
    return all(np.array_equal(mask[b], tril) for b in range(mask.shape[0]))


def _reference_fallback(x, mask, W_K_W, W_Q_W, W_V_W, W_pred_W):
    """Exact numpy mirror of the reference for non-causal masks."""
    x = np.asarray(x, np.float32)
    mask = np.asarray(mask, bool)
    WK, WQ, WV = _host_weights(W_K_W, W_Q_W, W_V_W, W_pred_W)
    M = N_CTX
    table = np.concatenate([
        np.array([-2.0], np.float32),
        (np.linspace(0.0, -float(M), M - 1) / M).astype(np.float32),
        (np.linspace(-float(M), 0.0, M) / M).astype(np.float32)])
    rel = (np.arange(M)[None, :] - np.arange(M)[:, None]) % (2 * M)
    bias = table[rel]
    out = np.zeros((BATCH, N_CTX, N_HEADS * D_HEAD), np.float32)
    for b in range(BATCH):
        for h in range(N_HEADS):
            k = x[b] @ WK[h]
            q = x[b] @ WQ[h]
            v = x[b] @ WV[h]
            pre = q @ k.T
            srow = np.where(mask[b], pre, 0.0).sum(-1)
            ms = srow / (srow + 1e-10)
            pre[:, 0] += np.maximum(1.0 - ms, 0.0)
            pos = np.log(pre + 1e-20) + bias
            masked = np.where(mask[b], pos, -1e30)
            masked = masked / 8.0
            masked -= masked.max(-1, keepdims=True)
            ex = np.exp(masked)
            attn = ex / ex.sum(-1, keepdims=True)
            out[b, :, h * 64:(h + 1) * 64] = attn @ v
    return out


def _run(inputs):
    from concourse.bass_utils import run_bass_kernel_spmd
    x = np.asarray(inputs["x"], np.float32)
    WK, WQ, WV = _host_weights(inputs["W_K_W"], inputs["W_Q_W"],
                               inputs["W_V_W"], inputs["W_pred_W"])
    nc = _get_nc()
    in_maps = _make_in_maps(x, WK, WQ, WV)
    res = run_bass_kernel_spmd(nc, in_maps, list(range(8)))
    out = np.empty((BATCH, N_CTX, N_HEADS * D_HEAD), np.float32)
    for c in range(8):
        b, hg = c // 2, c % 2
        out[b, :, hg * 256:(hg + 1) * 256] = res.results[c]["out"]
    return out, res


def kernel(**inputs) -> np.ndarray:
    if not _mask_is_tril(inputs["mask"]):
        return _reference_fallback(**inputs)
    out, _ = _run(inputs)
    return out
